# revision 7
# baseline (speedup 1.0000x reference)
"""Distributed Bass kernel for the quirky-softmax attention layer on 8 TRN2 NeuronCores.

Reference (N=4096, D=1024, fp32):
    Q = x@Wq + bq; K = x@Wk + bk; V = x@Wv + bv
    S = mask * (Q @ K.T)
    e = exp(S)
    out[i, j] = e[i, j] / rowsum(e)[j]       # quirky: denominator indexed by COLUMN
    return out @ V

Sharding: rows of x across 8 cores (512 rows each). Each core computes
K^T and V for its shard (fp8, scaled x32), all-gathers them in fp8 (half
the bytes of bf16), computes e^T with DoubleRow fp8 matmuls (scores
transposed: j on partitions, local i on free axis), local row-sums s[i]
via a ones-vector matmul, all-gathers s, then the output uses the
mean-subtraction identity (e = 1 + t):

    out[i,:] = sum_j u[j] V[j,:]  +  sum_j (u[j] t[i,j]) V[j,:],  u = 1/s

The first (dominant) term is a [D]-vector computed from local V in
bf16/fp32 and AllReduced (4 KB); the correction matmul runs in fp8
DoubleRow where quantization only touches the small t = e-1 signal.

Collective issue order: kt1, kt2, v1, s, v2, m — the tiny s gather
lands in the CC idle slot before the second V chunk so the output
matmul's first half starts early.
"""

import os
import numpy as np
import ml_dtypes

N = 4096
D = 1024
NC = 8
R = N // NC      # 512 rows per core
P = 128
KT = D // P      # 8 contraction subtiles
MT = D // P      # 8 output-feature tiles
JT = N // P      # 32 j tiles
IT = R // P      # 4 i tiles
H = 256          # kt all-gather chunk width (local j columns), 2 chunks
VH = 256         # v all-gather chunk rows, 2 chunks

LAST_EXEC_NS = None
LAST_RES = None

_cache = {}


def _try_install_ntff_hook():
    """Best-effort registration of the axon NTFF profiling hook (for tracing)."""
    import sys, types

    if "antenv.axon_hooks" in sys.modules:
        return True
    try:
        from trn_agent_boot.trn_boot import _ntff_profile_via_ctypes

        hook = _ntff_profile_via_ctypes("/opt/axon/libaxon_pjrt.so")
        if hook is None:
            return False
        mod = types.ModuleType("antenv.axon_hooks")
        mod.get_axon_ntff_profile_hook = lambda: hook
        mod.set_axon_ntff_profile_hook = lambda h: None
        sys.modules["antenv.axon_hooks"] = mod
        import antenv

        antenv.axon_hooks = mod

        # zero-egress container: the artifact upload would block on network
        from concourse import bass_utils

        bass_utils.upload_artifacts = lambda tmpdir: tmpdir
        return True
    except Exception:
        return False


def _install_neff_cache():
    """Content-keyed NEFF cache: identical BIR -> skip the multi-minute walrus compile."""
    import hashlib
    import shutil

    from concourse import bass2jax, bass_utils

    if getattr(bass_utils, "_neff_cache_installed", False):
        return
    orig = bass_utils.compile_bir_kernel

    def cached(bir_json, tmpdir, neff_name="file.neff"):
        import re

        key = re.sub(rb'"line": \d+', b'"line": 0', bir_json)
        key += os.environ.get("BASS_LDW_OPT", "0").encode()
        h = hashlib.sha256(key).hexdigest()[:24]
        cdir = "/tmp/bass_neff_cache"
        os.makedirs(cdir, exist_ok=True)
        cpath = os.path.join(cdir, h + ".neff")
        if os.path.exists(cpath):
            dst = os.path.join(tmpdir, neff_name)
            shutil.copy(cpath, dst)
            return dst
        p = orig(bir_json, tmpdir, neff_name)
        try:
            shutil.copy(p, cpath)
        except OSError:
            pass
        return p

    bass_utils.compile_bir_kernel = cached
    bass2jax.compile_bir_kernel = cached
    bass_utils._neff_cache_installed = True

    if os.environ.get("BASS_LDW_OPT", "0") == "1":
        orig_run = bass_utils.run_command

        def run_ldw(cmd, *a, **kw):
            cmd = [
                c.replace("--enable-ldw-opt=false", "--enable-ldw-opt=true")
                if isinstance(c, str) else c
                for c in cmd
            ]
            return orig_run(cmd, *a, **kw)

        bass_utils.run_command = run_ldw


def _build():
    import concourse.bacc as bacc
    import concourse.mybir as mybir
    import concourse.tile as tile

    f32 = mybir.dt.float32
    bf16 = mybir.dt.bfloat16
    f8 = mybir.dt.float8e4
    DR = mybir.MatmulPerfMode.DoubleRow
    Ident = mybir.ActivationFunctionType.Identity
    Exp = mybir.ActivationFunctionType.Exp
    RG = [list(range(NC))]

    nc = bacc.Bacc("TRN2", target_bir_lowering=False, debug=False, num_devices=NC)

    xt8 = nc.declare_dram_parameter("xt8", [D, R], f8, isOutput=False)      # 32*x^T
    xtb = nc.declare_dram_parameter("xtb", [D, R], bf16, isOutput=False)    # x^T
    maskT = nc.declare_dram_parameter("maskT", [N, R], bf16, isOutput=False)  # mask^T / 1024
    wq8 = nc.declare_dram_parameter("wq8", [D, D], f8, isOutput=False)      # 32*Wq
    wk8 = nc.declare_dram_parameter("wk8", [D, D], f8, isOutput=False)      # 32*Wk
    wvb = nc.declare_dram_parameter("wvb", [D, D], bf16, isOutput=False)
    bq32 = nc.declare_dram_parameter("bq32", [D], f32, isOutput=False)      # 32*bq
    bk32 = nc.declare_dram_parameter("bk32", [D], f32, isOutput=False)      # 32*bk
    bvb = nc.declare_dram_parameter("bvb", [P, D], f32, isOutput=False)     # bv broadcast
    outT = nc.declare_dram_parameter("outT", [D, R], f32, isOutput=True)

    with tile.TileContext(nc) as tc:
        with tc.tile_pool(name="dram", bufs=1, space="DRAM") as dram, \
             tc.tile_pool(name="const", bufs=1) as const:
            kt_in1 = dram.tile([D, H], f8)
            kt_ag1 = dram.tile([NC * D, H], f8, addr_space="Shared")
            kt_in2 = dram.tile([D, H], f8)
            kt_ag2 = dram.tile([NC * D, H], f8, addr_space="Shared")
            v_in1 = dram.tile([VH, D], f8)
            v_ag1 = dram.tile([NC * VH, D], f8, addr_space="Shared")
            v_in2 = dram.tile([VH, D], f8)
            v_ag2 = dram.tile([NC * VH, D], f8, addr_space="Shared")
            s_in = dram.tile([1, R], f32)
            s_ag = dram.tile([NC, R], f32, addr_space="Shared")
            m_in = dram.tile([1, D], f32)
            m_ar = dram.tile([1, D], f32)

            # ---- persistent SBUF ----
            qt_sb = const.tile([P, KT, R], f8)       # 32*Q^T
            kt_sb = const.tile([P, MT, R], f8)       # 32*K^T (local)
            v_sb = const.tile([P, IT, D], bf16)      # V (local, natural layout)
            et_sb = const.tile([P, JT, R], bf16)     # e^T = exp(mask*scores)^T
            tu_sb = const.tile([P, JT, R], f8)       # (e-1)*2^18/s[j]
            mask_sb = const.tile([P, JT, R], bf16)   # mask^T / 1024
            ones_sb = const.tile([P, 1], bf16)
            bk32_sb = const.tile([P, MT], f32)
            bq32_sb = const.tile([P, MT], f32)
            bv_sb = const.tile([P, D], f32)
            s_loc = const.tile([P, IT], f32)
            s_loc2 = const.tile([P, IT], f32)
            u_loc = const.tile([P, IT], f32)         # 4096/s local rows
            sr_sb = const.tile([P, JT], f32)
            sr2_sb = const.tile([P, JT], f32)
            r_sb = const.tile([P, JT], f32)          # 2^18/s[j]
            m_pt = const.tile([P, MT], f32)
            m_sb = const.tile([P, MT], f32)          # mean term m[d]

            nc.vector.memset(ones_sb[:], 1.0)

            # ---------------- projections ----------------
            with tc.tile_pool(name="wpool", bufs=1) as wpool, \
                 tc.tile_pool(name="proj_ps", bufs=4, space="PSUM") as proj_ps:
                wk_sb = wpool.tile([P, KT, D], f8)
                xt8_sb = wpool.tile([P, KT, R], f8)
                for k in range(KT):
                    nc.scalar.dma_start(wk_sb[:, k, :], wk8.ap()[k * P:(k + 1) * P, :])
                    nc.scalar.dma_start(xt8_sb[:, k, :], xt8.ap()[k * P:(k + 1) * P, :])
                nc.scalar.dma_start(bk32_sb[:], bk32.ap().rearrange("(m p) -> p m", p=P))
                nc.scalar.dma_start(bq32_sb[:], bq32.ap().rearrange("(m p) -> p m", p=P))
                nc.scalar.dma_start(bv_sb[:], bvb.ap())

                # K^T = 32*(x@Wk + bk)^T in fp8, via DoubleRow fp8 matmuls
                for m in range(MT):
                    ps = proj_ps.tile([P, R], f32, tag="ps", name=f"ps_k{m}")
                    for kk in range(KT // 2):
                        nc.tensor.matmul(
                            ps[:], wk_sb[:, 2 * kk:2 * kk + 2, m * P:(m + 1) * P],
                            xt8_sb[:, 2 * kk:2 * kk + 2, :],
                            start=(kk == 0), stop=(kk == KT // 2 - 1), perf_mode=DR,
                        )
                    nc.scalar.activation(
                        kt_sb[:, m, :], ps[:], Ident,
                        bias=bk32_sb[:, m:m + 1], scale=1.0 / 32.0,
                    )
                # K^T all-gather in two j-chunks (fp8)
                nc.gpsimd.dma_start(
                    kt_in1.rearrange("(m p) j -> p m j", p=P), kt_sb[:, :, 0:H]
                )
                nc.gpsimd.collective_compute(
                    "AllGather", mybir.AluOpType.bypass, replica_groups=RG,
                    ins=[kt_in1.opt()], outs=[kt_ag1.opt()],
                )
                nc.gpsimd.dma_start(
                    kt_in2.rearrange("(m p) j -> p m j", p=P), kt_sb[:, :, H:R]
                )
                nc.gpsimd.collective_compute(
                    "AllGather", mybir.AluOpType.bypass, replica_groups=RG,
                    ins=[kt_in2.opt()], outs=[kt_ag2.opt()],
                )

                # V (natural layout, bf16): lhsT = x^T tile, rhs = Wv tile
                wv_sb = wpool.tile([P, KT, D], bf16)
                xtb_sb = wpool.tile([P, KT, R], bf16)
                nc.scalar.dma_start(wv_sb[:], wvb.ap().rearrange("(k p) o -> p k o", p=P))
                nc.scalar.dma_start(xtb_sb[:], xtb.ap().rearrange("(k p) i -> p k i", p=P))
                v8_sb = wpool.tile([P, IT, D], f8)
                for it in range(IT):
                    for c2 in range(2):
                        ps = proj_ps.tile([P, 512], f32, tag="ps", name=f"ps_v{it}_{c2}")
                        for k in range(KT):
                            nc.tensor.matmul(
                                ps[:], xtb_sb[:, k, it * P:(it + 1) * P],
                                wv_sb[:, k, c2 * 512:(c2 + 1) * 512],
                                start=(k == 0), stop=(k == KT - 1),
                            )
                        nc.vector.tensor_add(
                            v_sb[:, it, c2 * 512:(c2 + 1) * 512], ps[:],
                            bv_sb[:, c2 * 512:(c2 + 1) * 512],
                        )
                    nc.scalar.mul(v8_sb[:, it, :], v_sb[:, it, :], 32.0)  # fp8 32*V
                nc.gpsimd.dma_start(
                    v_in1.rearrange("(t p) d -> p t d", p=P), v8_sb[:, 0:2, :]
                )
                nc.gpsimd.dma_start(
                    v_in2.rearrange("(t p) d -> p t d", p=P), v8_sb[:, 2:4, :]
                )
                nc.gpsimd.collective_compute(
                    "AllGather", mybir.AluOpType.bypass, replica_groups=RG,
                    ins=[v_in1.opt()], outs=[v_ag1.opt()],
                )

                # Q^T = 32*(x@Wq + bq)^T in fp8
                wq_sb = wpool.tile([P, KT, D], f8)
                nc.scalar.dma_start(wq_sb[:], wq8.ap().rearrange("(k p) o -> p k o", p=P))
                for m in range(MT):
                    ps = proj_ps.tile([P, R], f32, tag="ps", name=f"ps_q{m}")
                    for kk in range(KT // 2):
                        nc.tensor.matmul(
                            ps[:], wq_sb[:, 2 * kk:2 * kk + 2, m * P:(m + 1) * P],
                            xt8_sb[:, 2 * kk:2 * kk + 2, :],
                            start=(kk == 0), stop=(kk == KT // 2 - 1), perf_mode=DR,
                        )
                    nc.scalar.activation(
                        qt_sb[:, m, :], ps[:], Ident,
                        bias=bq32_sb[:, m:m + 1], scale=1.0 / 32.0,
                    )

                # mask prefetch into SBUF (hides in the launch-skew window)
                for g in range(JT // 4):
                    nc.scalar.dma_start(
                        mask_sb[:, 4 * g:4 * g + 4, :],
                        maskT.ap()[g * 512:(g + 1) * 512, :]
                        .rearrange("(t p) i -> p t i", p=P),
                    )

            # ---------------- scores^T + exp + rowsums + mean path ----------------
            with tc.tile_pool(name="ktp", bufs=3) as ktp, \
                 tc.tile_pool(name="mskp", bufs=3) as mskp, \
                 tc.tile_pool(name="sp2", bufs=1) as sp2, \
                 tc.tile_pool(name="sc_ps", bufs=4, space="PSUM") as sc_ps, \
                 tc.tile_pool(name="s1_ps", bufs=1, space="PSUM") as s1_ps, \
                 tc.tile_pool(name="m_ps", bufs=1, space="PSUM") as m_ps:
                s1 = s1_ps.tile([1, R], f32)

                # processing order: chunk1 (local j cols 0..255 of each rank's
                # slab = tiles 4c+0, 4c+1), then chunk2 (tiles 4c+2, 4c+3)
                seq = []
                for c in range(NC):
                    seq += [(4 * c + 0, 1, c, 0), (4 * c + 1, 1, c, P)]
                for c in range(NC):
                    seq += [(4 * c + 2, 2, c, 0), (4 * c + 3, 2, c, P)]

                def rowsum_mm(pos):
                    t = seq[pos][0]
                    nc.tensor.matmul(
                        s1[:], ones_sb[:], et_sb[:, t, :],
                        start=(pos == 0), stop=(pos == JT - 1),
                    )

                ktc = None
                for pos, (t, ch, c, koff) in enumerate(seq):
                    if koff == 0:
                        ktc = ktp.tile([P, KT, H], f8, tag="kt", name=f"ktc{ch}_{c}")
                        src = kt_ag1 if ch == 1 else kt_ag2
                        nc.sync.dma_start(
                            ktc[:],
                            src[c * D:(c + 1) * D, :]
                            .rearrange("(k p) j -> p k j", p=P),
                        )
                    ps = sc_ps.tile([P, R], f32, tag="ps", name=f"ps_s{t}")
                    for kk in range(KT // 2):
                        nc.tensor.matmul(
                            ps[:], ktc[:, 2 * kk:2 * kk + 2, koff:koff + P],
                            qt_sb[:, 2 * kk:2 * kk + 2, :],
                            start=(kk == 0), stop=(kk == KT // 2 - 1), perf_mode=DR,
                        )
                    # lag the rowsum matmul so PE never waits on ACT
                    if pos >= 2:
                        rowsum_mm(pos - 2)
                    msk = mskp.tile([P, R], f32, tag="msk", name=f"msk{t}")
                    nc.vector.tensor_mul(msk[:], ps[:], mask_sb[:, t, :])
                    nc.scalar.activation(et_sb[:, t, :], msk[:], Exp)
                rowsum_mm(JT - 2)
                rowsum_mm(JT - 1)

                # export local rowsums; CC order: (kt1, kt2, v1 already), s, v2, m
                s_sb = sp2.tile([1, R], f32, tag="s_sb")
                nc.vector.tensor_copy(s_sb[:], s1[:])
                nc.gpsimd.dma_start(s_in[:], s_sb[:])
                nc.gpsimd.collective_compute(
                    "AllGather", mybir.AluOpType.bypass, replica_groups=RG,
                    ins=[s_in.opt()], outs=[s_ag.opt()],
                )
                nc.gpsimd.collective_compute(
                    "AllGather", mybir.AluOpType.bypass, replica_groups=RG,
                    ins=[v_in2.opt()], outs=[v_ag2.opt()],
                )

                # mean term: m[d] = sum_{j local} (4096/s[j]) V[j,d], AllReduce
                nc.scalar.dma_start(s_loc[:], s_in.rearrange("r (t p) -> p (r t)", p=P))
                nc.vector.tensor_scalar_mul(s_loc2[:], s_loc[:], 2.0 ** -12)
                nc.vector.reciprocal(u_loc[:], s_loc2[:])      # 4096/s
                vus_sb = sp2.tile([P, IT, D], bf16, tag="vus")
                for it in range(IT):
                    nc.vector.tensor_scalar_mul(
                        vus_sb[:, it, :], v_sb[:, it, :], u_loc[:, it:it + 1]
                    )
                mp0 = m_ps.tile([1, 512], f32, name="mp0")
                mp1 = m_ps.tile([1, 512], f32, name="mp1")
                for it in range(IT):
                    nc.tensor.matmul(
                        mp0[:], ones_sb[:], vus_sb[:, it, 0:512],
                        start=(it == 0), stop=(it == IT - 1),
                    )
                for it in range(IT):
                    nc.tensor.matmul(
                        mp1[:], ones_sb[:], vus_sb[:, it, 512:1024],
                        start=(it == 0), stop=(it == IT - 1),
                    )
                m_flat = sp2.tile([1, D], f32, tag="m_flat")
                nc.vector.tensor_copy(m_flat[:, 0:512], mp0[:])
                nc.vector.tensor_copy(m_flat[:, 512:1024], mp1[:])
                nc.scalar.dma_start(m_in[:], m_flat[:])
                # (the m AllReduce is issued later, after the tu ops, so its
                # input wait does not stall the gpsimd queue; CC arrival order
                # stays s -> v2 -> m on every core)

                # r_sb = 2^18/s[j] in (p, t) layout
                nc.scalar.dma_start(sr_sb[:], s_ag.rearrange("r (tt p) -> p (r tt)", p=P))
                nc.vector.tensor_scalar_mul(sr2_sb[:], sr_sb[:], 2.0 ** -18)
                nc.vector.reciprocal(r_sb[:], sr2_sb[:])

            # ---------------- out^T correction + mean add ----------------
            with tc.tile_pool(name="vp", bufs=3) as vp, \
                 tc.tile_pool(name="op", bufs=1) as op, \
                 tc.tile_pool(name="out_ps", bufs=1, space="PSUM") as out_ps:
                pso = [out_ps.tile([P, R], f32, name=f"pso{m}") for m in range(MT)]
                sub = mybir.AluOpType.subtract
                mult = mybir.AluOpType.mult
                add = mybir.AluOpType.add
                pairs = [(c, 0) for c in range(NC)] + [(c, 1) for c in range(NC)]
                NPAIR = len(pairs)
                for pi, (c, pr) in enumerate(pairs):
                    t0 = 4 * c + 2 * pr
                    for tt in (t0, t0 + 1):
                        eng = nc.vector if tt % 2 == 0 else nc.gpsimd
                        eng.tensor_scalar(
                            out=tu_sb[:, tt, :], in0=et_sb[:, tt, :],
                            scalar1=1.0, scalar2=r_sb[:, tt:tt + 1],
                            op0=sub, op1=mult,
                        )
                    vt = vp.tile([P, 2, D], f8, tag="v", name=f"vt{pi}")
                    vag = v_ag1 if pr == 0 else v_ag2
                    nc.sync.dma_start(
                        vt[:],
                        vag[c * VH:(c + 1) * VH, :]
                        .rearrange("(t p) d -> p t d", p=P),
                    )
                    for m in range(MT):
                        nc.tensor.matmul(
                            pso[m][:], vt[:, :, m * P:(m + 1) * P],
                            tu_sb[:, t0:t0 + 2, :],
                            start=(pi == 0), stop=(pi == NPAIR - 1), perf_mode=DR,
                        )
                nc.gpsimd.collective_compute(
                    "AllReduce", mybir.AluOpType.add, replica_groups=RG,
                    ins=[m_in.opt()], outs=[m_ar.opt()],
                )
                # m_sb = m_ar * 2^-12  (on ACT: DVE is busy with tu scaling)
                nc.scalar.dma_start(m_pt[:], m_ar.rearrange("r (m p) -> p (r m)", p=P))
                nc.scalar.mul(m_sb[:], m_pt[:], 2.0 ** -12)
                ot_sb = op.tile([P, MT, R], f32)
                for m in range(MT):
                    nc.vector.tensor_scalar(
                        out=ot_sb[:, m, :], in0=pso[m][:],
                        scalar1=2.0 ** -23, scalar2=m_sb[:, m:m + 1],
                        op0=mult, op1=add,
                    )
                    nc.sync.dma_start(outT.ap()[m * P:(m + 1) * P, :], ot_sb[:, m, :])

    nc.finalize()
    return nc


def _get_nc():
    if "nc" not in _cache:
        _cache["nc"] = _build()
    return _cache["nc"]


def kernel(x, mask, Wq, bq, Wk, bk, Wv, bv):
    global LAST_EXEC_NS
    _install_neff_cache()
    from concourse.bass_utils import run_bass_kernel_spmd

    bf = ml_dtypes.bfloat16
    f8 = ml_dtypes.float8_e4m3fn
    x = np.asarray(x, dtype=np.float32)
    mask = np.asarray(mask, dtype=np.float32)
    wq8 = (32.0 * np.asarray(Wq, dtype=np.float32)).astype(f8)
    wk8 = (32.0 * np.asarray(Wk, dtype=np.float32)).astype(f8)
    wvb = np.asarray(Wv, dtype=np.float32).astype(bf)
    bq32 = 32.0 * np.asarray(bq, dtype=np.float32)
    bk32 = 32.0 * np.asarray(bk, dtype=np.float32)
    bvb = np.ascontiguousarray(
        np.broadcast_to(np.asarray(bv, dtype=np.float32), (P, D))
    )
    mask_s = (mask * (1.0 / 1024.0)).astype(bf)

    in_maps = []
    for c in range(NC):
        rows = slice(c * R, (c + 1) * R)
        xT = np.ascontiguousarray(x[rows, :].T)
        xTb = xT.astype(bf)
        in_maps.append({
            "xt8": (32.0 * xTb.astype(np.float32)).astype(f8),
            "xtb": xTb,
            "maskT": np.ascontiguousarray(mask_s[rows, :].T),
            "wq8": wq8, "wk8": wk8, "wvb": wvb,
            "bq32": bq32, "bk32": bk32, "bvb": bvb,
        })

    nc = _get_nc()
    trace = os.environ.get("BASS_KERNEL_TRACE", "0") == "1"
    if trace:
        trace = _try_install_ntff_hook()
    res = run_bass_kernel_spmd(
        nc, in_maps, core_ids=list(range(NC)), trace=trace,
        **({"trace_cores": [0]} if trace else {}),
    )
    LAST_EXEC_NS = res.exec_time_ns
    globals()["LAST_RES"] = res
    out = np.concatenate(
        [res.results[c]["outT"].T for c in range(NC)], axis=0
    ).astype(np.float32)
    return out


# revision 17
# speedup vs baseline: 1.6677x; 1.6677x over previous
"""Distributed Bass kernel for the quirky-softmax attention layer on 8 TRN2 NeuronCores.

Reference (N=4096, D=1024, fp32):
    Q = x@Wq + bq; K = x@Wk + bk; V = x@Wv + bv
    S = mask * (Q @ K.T)
    e = exp(S)
    out[i, j] = e[i, j] / rowsum(e)[j]       # quirky: denominator indexed by COLUMN
    return out @ V

Sharding: rows of x across 8 cores (512 rows each). Each core computes
K^T and V for its shard (fp8, scaled x32), all-gathers them in fp8 (half
the bytes of bf16), computes e^T with DoubleRow fp8 matmuls (scores
transposed: j on partitions, local i on free axis), local row-sums s[i]
via a ones-vector matmul, all-gathers s, then the output uses the
mean-subtraction identity (e = 1 + t):

    out[i,:] = sum_j u[j] V[j,:]  +  sum_j (u[j] t[i,j]) V[j,:],  u = 1/s

The first (dominant) term is a [D]-vector computed from local V in
bf16/fp32 and AllReduced (4 KB); the correction matmul runs in fp8
DoubleRow where quantization only touches the small t = e-1 signal.

Collective issue order: kt1, kt2, v1, s, v2, m — the tiny s gather
lands in the CC idle slot before the second V chunk so the output
matmul's first half starts early.
"""

import os
import numpy as np
import ml_dtypes

N = 4096
D = 1024
NC = 8
R = N // NC      # 512 rows per core
P = 128
KT = D // P      # 8 contraction subtiles
MT = D // P      # 8 output-feature tiles
JT = N // P      # 32 j tiles
IT = R // P      # 4 i tiles
H = 256          # kt all-gather chunk width (local j columns), 2 chunks
VH = 256         # v all-gather chunk rows, 2 chunks

LAST_EXEC_NS = None
LAST_RES = None

_cache = {}


def _try_install_ntff_hook():
    """Best-effort registration of the axon NTFF profiling hook (for tracing)."""
    import sys, types

    if "antenv.axon_hooks" in sys.modules:
        return True
    try:
        from trn_agent_boot.trn_boot import _ntff_profile_via_ctypes

        hook = _ntff_profile_via_ctypes("/opt/axon/libaxon_pjrt.so")
        if hook is None:
            return False
        mod = types.ModuleType("antenv.axon_hooks")
        mod.get_axon_ntff_profile_hook = lambda: hook
        mod.set_axon_ntff_profile_hook = lambda h: None
        sys.modules["antenv.axon_hooks"] = mod
        import antenv

        antenv.axon_hooks = mod

        # zero-egress container: the artifact upload would block on network
        from concourse import bass_utils

        bass_utils.upload_artifacts = lambda tmpdir: tmpdir
        return True
    except Exception:
        return False


def _install_neff_cache():
    """Content-keyed NEFF cache: identical BIR -> skip the multi-minute walrus compile."""
    import hashlib
    import shutil

    from concourse import bass2jax, bass_utils

    if getattr(bass_utils, "_neff_cache_installed", False):
        return
    orig = bass_utils.compile_bir_kernel

    def cached(bir_json, tmpdir, neff_name="file.neff"):
        import re

        key = re.sub(rb'"line": \d+', b'"line": 0', bir_json)
        key += os.environ.get("BASS_LDW_OPT", "0").encode()
        h = hashlib.sha256(key).hexdigest()[:24]
        cdir = "/tmp/bass_neff_cache"
        os.makedirs(cdir, exist_ok=True)
        cpath = os.path.join(cdir, h + ".neff")
        if os.path.exists(cpath):
            dst = os.path.join(tmpdir, neff_name)
            shutil.copy(cpath, dst)
            return dst
        p = orig(bir_json, tmpdir, neff_name)
        try:
            shutil.copy(p, cpath)
        except OSError:
            pass
        return p

    bass_utils.compile_bir_kernel = cached
    bass2jax.compile_bir_kernel = cached
    bass_utils._neff_cache_installed = True

    if os.environ.get("BASS_LDW_OPT", "0") == "1":
        orig_run = bass_utils.run_command

        def run_ldw(cmd, *a, **kw):
            cmd = [
                c.replace("--enable-ldw-opt=false", "--enable-ldw-opt=true")
                if isinstance(c, str) else c
                for c in cmd
            ]
            return orig_run(cmd, *a, **kw)

        bass_utils.run_command = run_ldw


def _build():
    import concourse.bacc as bacc
    import concourse.mybir as mybir
    import concourse.tile as tile

    f32 = mybir.dt.float32
    bf16 = mybir.dt.bfloat16
    f8 = mybir.dt.float8e4
    DR = mybir.MatmulPerfMode.DoubleRow
    Ident = mybir.ActivationFunctionType.Identity
    Exp = mybir.ActivationFunctionType.Exp
    RG = [list(range(NC))]

    nc = bacc.Bacc("TRN2", target_bir_lowering=False, debug=False, num_devices=NC)

    xt8 = nc.declare_dram_parameter("xt8", [D, R], f8, isOutput=False)      # 32*x^T
    xtb = nc.declare_dram_parameter("xtb", [D, R], bf16, isOutput=False)    # x^T
    maskT = nc.declare_dram_parameter("maskT", [N, R], bf16, isOutput=False)  # mask^T / 1024
    wq8 = nc.declare_dram_parameter("wq8", [D, D], f8, isOutput=False)      # 32*Wq
    wk8 = nc.declare_dram_parameter("wk8", [D, D], f8, isOutput=False)      # 32*Wk
    wvb = nc.declare_dram_parameter("wvb", [D, D], bf16, isOutput=False)
    bq32 = nc.declare_dram_parameter("bq32", [D], f32, isOutput=False)      # 32*bq
    bk32 = nc.declare_dram_parameter("bk32", [D], f32, isOutput=False)      # 32*bk
    bvb = nc.declare_dram_parameter("bvb", [P, D], f32, isOutput=False)     # bv broadcast
    outT = nc.declare_dram_parameter("outT", [D, R], f32, isOutput=True)

    with tile.TileContext(nc) as tc:
        with tc.tile_pool(name="dram", bufs=1, space="DRAM") as dram, \
             tc.tile_pool(name="const", bufs=1) as const:
            kt_in = dram.tile([D, R], f8)
            kt_ag = dram.tile([NC * D, R], f8, addr_space="Shared")
            v_in = dram.tile([R, D], f8)
            v_ag = dram.tile([N, D], f8, addr_space="Shared")
            s_in = dram.tile([1, R], f32)
            s_ag = dram.tile([NC, R], f32, addr_space="Shared")
            m_in = dram.tile([1, D], f32)
            m_ar = dram.tile([1, D], f32)

            # ---- persistent SBUF ----
            qt_sb = const.tile([P, KT, R], f8)       # 32*Q^T
            kt_sb = const.tile([P, MT, R], f8)       # 32*K^T (local)
            v_sb = const.tile([P, IT, D], bf16)      # V (local, natural layout)
            et_sb = const.tile([P, JT, R], bf16)     # e^T = exp(mask*scores)^T
            tu_sb = const.tile([P, JT, R], f8)       # (e-1)*2^18/s[j]
            mask_sb = const.tile([P, JT, R], bf16)   # mask^T / 1024
            ones_sb = const.tile([P, 1], bf16)
            bk32_sb = const.tile([P, MT], f32)
            bq32_sb = const.tile([P, MT], f32)
            bv_sb = const.tile([P, D], f32)
            s_loc = const.tile([P, IT], f32)
            s_loc2 = const.tile([P, IT], f32)
            u_loc = const.tile([P, IT], f32)         # 4096/s local rows
            sr_sb = const.tile([P, JT], f32)
            sr2_sb = const.tile([P, JT], f32)
            r_sb = const.tile([P, JT], f32)          # 2^18/s[j]
            negr_sb = const.tile([P, JT], f32)       # -2^18/s[j]
            m_pt = const.tile([P, MT], f32)
            m_sb = const.tile([P, MT], f32)          # mean term m[d]

            nc.vector.memset(ones_sb[:], 1.0)

            # ---------------- projections ----------------
            with tc.tile_pool(name="wpool", bufs=1) as wpool, \
                 tc.tile_pool(name="proj_ps", bufs=4, space="PSUM") as proj_ps:
                wk_sb = wpool.tile([P, KT, D], f8)
                xt8_sb = wpool.tile([P, KT, R], f8)
                for k in range(KT):
                    nc.scalar.dma_start(wk_sb[:, k, :], wk8.ap()[k * P:(k + 1) * P, :])
                    nc.scalar.dma_start(xt8_sb[:, k, :], xt8.ap()[k * P:(k + 1) * P, :])
                nc.scalar.dma_start(bk32_sb[:], bk32.ap().rearrange("(m p) -> p m", p=P))
                nc.scalar.dma_start(bq32_sb[:], bq32.ap().rearrange("(m p) -> p m", p=P))
                nc.scalar.dma_start(bv_sb[:], bvb.ap())

                # K^T = 32*(x@Wk + bk)^T in fp8, via DoubleRow fp8 matmuls
                for m in range(MT):
                    ps = proj_ps.tile([P, R], f32, tag="ps", name=f"ps_k{m}")
                    for kk in range(KT // 2):
                        nc.tensor.matmul(
                            ps[:], wk_sb[:, 2 * kk:2 * kk + 2, m * P:(m + 1) * P],
                            xt8_sb[:, 2 * kk:2 * kk + 2, :],
                            start=(kk == 0), stop=(kk == KT // 2 - 1), perf_mode=DR,
                        )
                    nc.scalar.activation(
                        kt_sb[:, m, :], ps[:], Ident,
                        bias=bk32_sb[:, m:m + 1], scale=1.0 / 32.0,
                    )
                # K^T all-gather (single fp8 chunk: per-CC fixed cost ~13us
                # dominates at these sizes, so fewer, bigger collectives win)
                nc.gpsimd.dma_start(
                    kt_in.rearrange("(m p) j -> p m j", p=P), kt_sb[:]
                )
                nc.gpsimd.collective_compute(
                    "AllGather", mybir.AluOpType.bypass, replica_groups=RG,
                    ins=[kt_in.opt()], outs=[kt_ag.opt()],
                )

                # V (natural layout, bf16): lhsT = x^T tile, rhs = Wv tile
                wv_sb = wpool.tile([P, KT, D], bf16)
                xtb_sb = wpool.tile([P, KT, R], bf16)
                nc.scalar.dma_start(wv_sb[:], wvb.ap().rearrange("(k p) o -> p k o", p=P))
                nc.scalar.dma_start(xtb_sb[:], xtb.ap().rearrange("(k p) i -> p k i", p=P))
                v8_sb = wpool.tile([P, IT, D], f8)
                for it in range(IT):
                    for c2 in range(2):
                        ps = proj_ps.tile([P, 512], f32, tag="ps", name=f"ps_v{it}_{c2}")
                        for k in range(KT):
                            nc.tensor.matmul(
                                ps[:], xtb_sb[:, k, it * P:(it + 1) * P],
                                wv_sb[:, k, c2 * 512:(c2 + 1) * 512],
                                start=(k == 0), stop=(k == KT - 1),
                            )
                        nc.vector.tensor_add(
                            v_sb[:, it, c2 * 512:(c2 + 1) * 512], ps[:],
                            bv_sb[:, c2 * 512:(c2 + 1) * 512],
                        )
                    nc.scalar.mul(v8_sb[:, it, :], v_sb[:, it, :], 32.0)  # fp8 32*V
                nc.gpsimd.dma_start(
                    v_in.rearrange("(t p) d -> p t d", p=P), v8_sb[:]
                )
                nc.gpsimd.collective_compute(
                    "AllGather", mybir.AluOpType.bypass, replica_groups=RG,
                    ins=[v_in.opt()], outs=[v_ag.opt()],
                )

                # Q^T = 32*(x@Wq + bq)^T in fp8
                wq_sb = wpool.tile([P, KT, D], f8)
                nc.scalar.dma_start(wq_sb[:], wq8.ap().rearrange("(k p) o -> p k o", p=P))
                for m in range(MT):
                    ps = proj_ps.tile([P, R], f32, tag="ps", name=f"ps_q{m}")
                    for kk in range(KT // 2):
                        nc.tensor.matmul(
                            ps[:], wq_sb[:, 2 * kk:2 * kk + 2, m * P:(m + 1) * P],
                            xt8_sb[:, 2 * kk:2 * kk + 2, :],
                            start=(kk == 0), stop=(kk == KT // 2 - 1), perf_mode=DR,
                        )
                    nc.scalar.activation(
                        qt_sb[:, m, :], ps[:], Ident,
                        bias=bq32_sb[:, m:m + 1], scale=1.0 / 32.0,
                    )

                # mask prefetch into SBUF (hides in the launch-skew window)
                for g in range(JT // 4):
                    nc.scalar.dma_start(
                        mask_sb[:, 4 * g:4 * g + 4, :],
                        maskT.ap()[g * 512:(g + 1) * 512, :]
                        .rearrange("(t p) i -> p t i", p=P),
                    )

            # ---------------- scores^T + exp + rowsums + mean path ----------------
            with tc.tile_pool(name="ktp", bufs=3) as ktp, \
                 tc.tile_pool(name="mskp", bufs=3) as mskp, \
                 tc.tile_pool(name="sp2", bufs=1) as sp2, \
                 tc.tile_pool(name="sc_ps", bufs=4, space="PSUM") as sc_ps, \
                 tc.tile_pool(name="s1_ps", bufs=1, space="PSUM") as s1_ps, \
                 tc.tile_pool(name="m_ps", bufs=1, space="PSUM") as m_ps:
                s1 = s1_ps.tile([1, R], f32)

                # per-rank slabs of 4 j-tiles each
                seq = [(4 * c + tl, c, tl * P) for c in range(NC) for tl in range(IT)]

                def rowsum_mm(pos):
                    t = seq[pos][0]
                    nc.tensor.matmul(
                        s1[:], ones_sb[:], et_sb[:, t, :],
                        start=(pos == 0), stop=(pos == JT - 1),
                    )

                ktc = None
                for pos, (t, c, koff) in enumerate(seq):
                    if koff == 0:
                        ktc = ktp.tile([P, KT, R], f8, tag="kt", name=f"ktc_{c}")
                        nc.sync.dma_start(
                            ktc[:],
                            kt_ag[c * D:(c + 1) * D, :]
                            .rearrange("(k p) j -> p k j", p=P),
                        )
                    ps = sc_ps.tile([P, R], f32, tag="ps", name=f"ps_s{t}")
                    for kk in range(KT // 2):
                        nc.tensor.matmul(
                            ps[:], ktc[:, 2 * kk:2 * kk + 2, koff:koff + P],
                            qt_sb[:, 2 * kk:2 * kk + 2, :],
                            start=(kk == 0), stop=(kk == KT // 2 - 1), perf_mode=DR,
                        )
                    # lag the rowsum matmul so PE never waits on ACT
                    if pos >= 2:
                        rowsum_mm(pos - 2)
                    msk = mskp.tile([P, R], f32, tag="msk", name=f"msk{t}")
                    nc.vector.tensor_mul(msk[:], ps[:], mask_sb[:, t, :])
                    nc.scalar.activation(et_sb[:, t, :], msk[:], Exp)
                rowsum_mm(JT - 2)
                rowsum_mm(JT - 1)

                # export local rowsums; CC order: (kt1, kt2, v1 already), s, v2, m
                s_sb = sp2.tile([1, R], f32, tag="s_sb")
                nc.vector.tensor_copy(s_sb[:], s1[:])
                nc.gpsimd.dma_start(s_in[:], s_sb[:])
                nc.gpsimd.collective_compute(
                    "AllGather", mybir.AluOpType.bypass, replica_groups=RG,
                    ins=[s_in.opt()], outs=[s_ag.opt()],
                )

                # mean term: m[d] = sum_{j local} (4096/s[j]) V[j,d], AllReduce
                nc.scalar.dma_start(s_loc[:], s_in.rearrange("r (t p) -> p (r t)", p=P))
                nc.vector.tensor_scalar_mul(s_loc2[:], s_loc[:], 2.0 ** -12)
                nc.vector.reciprocal(u_loc[:], s_loc2[:])      # 4096/s
                vus_sb = sp2.tile([P, IT, D], bf16, tag="vus")
                for it in range(IT):
                    nc.vector.tensor_scalar_mul(
                        vus_sb[:, it, :], v_sb[:, it, :], u_loc[:, it:it + 1]
                    )
                mp0 = m_ps.tile([1, 512], f32, name="mp0")
                mp1 = m_ps.tile([1, 512], f32, name="mp1")
                for it in range(IT):
                    nc.tensor.matmul(
                        mp0[:], ones_sb[:], vus_sb[:, it, 0:512],
                        start=(it == 0), stop=(it == IT - 1),
                    )
                for it in range(IT):
                    nc.tensor.matmul(
                        mp1[:], ones_sb[:], vus_sb[:, it, 512:1024],
                        start=(it == 0), stop=(it == IT - 1),
                    )
                m_flat = sp2.tile([1, D], f32, tag="m_flat")
                nc.vector.tensor_copy(m_flat[:, 0:512], mp0[:])
                nc.vector.tensor_copy(m_flat[:, 512:1024], mp1[:])
                nc.scalar.dma_start(m_in[:], m_flat[:])
                # (the m AllReduce is issued later, after the tu ops, so its
                # input wait does not stall the gpsimd queue; CC arrival order
                # stays s -> v2 -> m on every core)

                # r_sb = 2^18/s[j] in (p, t) layout; negr for the ACT bias
                nc.scalar.dma_start(sr_sb[:], s_ag.rearrange("r (tt p) -> p (r tt)", p=P))
                nc.vector.tensor_scalar_mul(sr2_sb[:], sr_sb[:], 2.0 ** -18)
                nc.vector.reciprocal(r_sb[:], sr2_sb[:])
                nc.vector.tensor_scalar_mul(negr_sb[:], r_sb[:], -1.0)

            # ---------------- out^T correction + mean add ----------------
            with tc.tile_pool(name="vp", bufs=3) as vp, \
                 tc.tile_pool(name="op", bufs=1) as op, \
                 tc.tile_pool(name="out_ps", bufs=1, space="PSUM") as out_ps:
                pso = [out_ps.tile([P, R], f32, name=f"pso{m}") for m in range(MT)]
                nc.gpsimd.collective_compute(
                    "AllReduce", mybir.AluOpType.add, replica_groups=RG,
                    ins=[m_in.opt()], outs=[m_ar.opt()],
                )
                NPAIR = JT // 2
                for pi in range(NPAIR):
                    t0 = 2 * pi
                    for tt in (t0, t0 + 1):
                        # tu = (e-1)*r = r*e + (-r), on ACT: the DVE two-op
                        # tensor_scalar path measures ~9us/tile vs ~0.9us here
                        nc.scalar.activation(
                            tu_sb[:, tt, :], et_sb[:, tt, :], Ident,
                            bias=negr_sb[:, tt:tt + 1], scale=r_sb[:, tt:tt + 1],
                        )
                    vt = vp.tile([P, 2, D], f8, tag="v", name=f"vt{pi}")
                    nc.sync.dma_start(
                        vt[:],
                        v_ag[t0 * P:(t0 + 2) * P, :]
                        .rearrange("(t p) d -> p t d", p=P),
                    )
                    for m in range(MT):
                        nc.tensor.matmul(
                            pso[m][:], vt[:, :, m * P:(m + 1) * P],
                            tu_sb[:, t0:t0 + 2, :],
                            start=(pi == 0), stop=(pi == NPAIR - 1), perf_mode=DR,
                        )
                # m_sb = m_ar * 2^-12 and the final mean-add, both on ACT
                nc.scalar.dma_start(m_pt[:], m_ar.rearrange("r (m p) -> p (r m)", p=P))
                nc.scalar.mul(m_sb[:], m_pt[:], 2.0 ** -12)
                ot_sb = op.tile([P, MT, R], f32)
                for m in range(MT):
                    nc.scalar.activation(
                        ot_sb[:, m, :], pso[m][:], Ident,
                        bias=m_sb[:, m:m + 1], scale=2.0 ** -23,
                    )
                    nc.sync.dma_start(outT.ap()[m * P:(m + 1) * P, :], ot_sb[:, m, :])

    nc.finalize()
    return nc


def _get_nc():
    if "nc" not in _cache:
        _cache["nc"] = _build()
    return _cache["nc"]


def kernel(x, mask, Wq, bq, Wk, bk, Wv, bv):
    global LAST_EXEC_NS
    _install_neff_cache()
    from concourse.bass_utils import run_bass_kernel_spmd

    bf = ml_dtypes.bfloat16
    f8 = ml_dtypes.float8_e4m3fn
    x = np.asarray(x, dtype=np.float32)
    mask = np.asarray(mask, dtype=np.float32)
    wq8 = (32.0 * np.asarray(Wq, dtype=np.float32)).astype(f8)
    wk8 = (32.0 * np.asarray(Wk, dtype=np.float32)).astype(f8)
    wvb = np.asarray(Wv, dtype=np.float32).astype(bf)
    bq32 = 32.0 * np.asarray(bq, dtype=np.float32)
    bk32 = 32.0 * np.asarray(bk, dtype=np.float32)
    bvb = np.ascontiguousarray(
        np.broadcast_to(np.asarray(bv, dtype=np.float32), (P, D))
    )
    mask_s = (mask * (1.0 / 1024.0)).astype(bf)

    in_maps = []
    for c in range(NC):
        rows = slice(c * R, (c + 1) * R)
        xT = np.ascontiguousarray(x[rows, :].T)
        xTb = xT.astype(bf)
        in_maps.append({
            "xt8": (32.0 * xTb.astype(np.float32)).astype(f8),
            "xtb": xTb,
            "maskT": np.ascontiguousarray(mask_s[rows, :].T),
            "wq8": wq8, "wk8": wk8, "wvb": wvb,
            "bq32": bq32, "bk32": bk32, "bvb": bvb,
        })

    nc = _get_nc()
    trace = os.environ.get("BASS_KERNEL_TRACE", "0") == "1"
    if trace:
        trace = _try_install_ntff_hook()
    res = run_bass_kernel_spmd(
        nc, in_maps, core_ids=list(range(NC)), trace=trace,
        **({"trace_cores": [0]} if trace else {}),
    )
    LAST_EXEC_NS = res.exec_time_ns
    globals()["LAST_RES"] = res
    out = np.concatenate(
        [res.results[c]["outT"].T for c in range(NC)], axis=0
    ).astype(np.float32)
    return out
